# revision 22
# baseline (speedup 1.0000x reference)
"""Betti-matching surrogate loss kernel for Trainium2 (8 NeuronCores).

Computes mean((probs - one_hot(gt_mask))^2) where gt_mask values are
{0,1,2} with ignore_index 2 mapped to class 0 (so class = (gt_mask == 1)).

Identity used (t := (m==1) in {0,1}):

    loss * N = sum((p0-1)^2) + sum(p1^2) + 2*sum(t * (p0 - p1))

HBM traffic is the roofline for this problem, so the host narrows
dtypes while sharding: probs f32 -> bf16 (device compute is bf16
anyway; the loss shifts ~5e-5 relative), gt_mask int32 -> int8
(lossless). Per-core bytes drop 24 MiB -> 10 MiB. Note the DMA engines
charge OUTPUT bytes, so fp8-with-cast-on-DMA does not beat bf16 here
(measured); 10 MiB of SBUF-side bytes (~29 us) is the floor for ops
that need 2-byte operands.

Engine split, chosen from measured rates (ACT pass 13.7us/plane any
dtype, DVE tensor_tensor 2x 8.5us/plane, DVE scalar_tensor_tensor 1x
but fused compare+mult+accumulate in one pass):

  ACT: acc0 = Square(1-p0) accumulate; acc1 = Square(p1) accumulate
       (all but one 4096-wide chunk)
  DVE: q = p0-p1 (2x); acc2 = sum((m==1)*q) via one fused
       scalar_tensor_tensor pass reading the int8 mask directly;
       p1*p1 for the offloaded chunk
  PE : ones-matmul reduction of the offloaded p1^2 chunk into PSUM

All input DMAs are hardware-DGE on the Sync queue (software DGE via
gpsimd measured slower per byte and its hoisted transfers stall the
init barrier). The first chunks' DMAs and the activation-table warmup
are hoisted into the entry block so the stream and the 1.3us Square
table load run during the framework preamble. Engine busy lands at
~30.5us each for ACT/DVE with the 10 MiB stream at ~29us; measured
span is that plus ~6us fixed preamble, ~2.5us end-block, and a few us
of pipeline fill/drain.

Sharding: core k = (b, g) with b = k // 4, g = k % 4 owns
probs[b, :, 8g:8g+8, :, :] and gt_mask[b, 8g:8g+8, :, :] — contiguous
views of the dtype-narrowed full inputs. Host reduces partials in f64.
"""

import os

import numpy as np

import concourse.bass as bass
import concourse.mybir as mybir
from concourse.bass_utils import run_bass_kernel_spmd
from concourse.tile import TileContext


import bass_rust


def split_multiwait_instructions(nc):
    """The walrus build in this image rejects any instruction carrying more
    than one sync wait ("Too many sync wait commands"). Tile's semaphore
    assignment freely attaches several. Hoist all but the last wait of each
    instruction onto injected same-engine NoOps placed directly before it —
    engine streams execute in order, so the waits still all complete before
    the real instruction issues."""
    k = 0
    for f in nc.m.functions:
        for bb in f.blocks:
            insts = bb.instructions
            out, changed = [], False
            for inst in insts:
                si = inst.sync_info
                if si is not None and si.on_wait and len(si.on_wait) > 1:
                    SI = type(si)
                    waits = list(si.on_wait)
                    for w in waits[:-1]:
                        nop = bass_rust.InstNoOp(
                            name=f"waitsplit-{k}",
                            engine=inst.engine,
                            sync_info=SI(on_wait=[w], on_update=[]),
                        )
                        k += 1
                        nc.register_instruction(nop)
                        out.append(nop)
                    inst.sync_info = SI(
                        on_wait=[waits[-1]], on_update=list(si.on_update)
                    )
                    changed = True
                out.append(inst)
            if changed:
                bb.instructions = out

def hoist_leading_dmas(nc, max_hoist=6):
    """Launch the input stream during the framework preamble: move the
    leading wait-free Sync-queue DMACopy instructions out of the body
    block and into the entry block, ahead of the init-barrier Drain.
    The SP sequencer dispatches them asynchronously before joining the
    barrier, so the transfers overlap the const-memset/barrier preamble.
    Only hardware-DGE (SP) DMAs are eligible: a gpsimd software-DGE DMA
    in main stalls the init barrier's Drain until the transfer itself
    completes (measured 7.7us). Also hoists the leading wait-free
    Activation (the Square-table warmup) so the 1.3us table load runs
    during the preamble."""
    f = nc.m.functions[0]
    blocks = {bb.name: bb for bb in f.blocks}
    body = next(
        (bb for bb in f.blocks if "tile_context" in bb.name
         and not bb.name.endswith("_end")),
        None,
    )
    main = blocks.get("main")
    if body is None or main is None:
        return
    hoist = []
    n_dma = 0
    for inst in body.instructions:
        tn = type(inst).__name__
        has_wait = inst.sync_info is not None and inst.sync_info.on_wait
        if tn == "InstDMACopy" and str(inst.engine) in ("EngineType.SP", "SP"):
            if has_wait or n_dma >= max_hoist:
                break
            hoist.append(inst)
            n_dma += 1
        elif tn == "InstActivation" and not has_wait and not hoist:
            hoist.append(inst)
        elif tn in ("InstDMACopy", "InstNoOp"):
            continue
        else:
            break
    if not hoist:
        return
    names = {i.name for i in hoist}
    body.instructions = [i for i in body.instructions if i.name not in names]
    mi = main.instructions
    # Insert right after the entry InstCall: the SP sequencer then issues
    # the DMAs before its register moves, pulling the stream start forward.
    cut = 1 if mi and type(mi[0]).__name__ == "InstCall" else 0
    main.instructions = mi[:cut] + hoist + mi[cut:]


def overlap_final_store(nc, n_stores=2):
    """Take the output-store DMAs' HBM-write receipt off the critical path.
    The kernel tail otherwise serializes: last compute -> store DMA issue ->
    ~1.4us sem-update receipt -> end-block waits -> barriers -> epilogue.
    Nothing in the program consumes the stores' data or slots, and the
    wrapper epilogue (~7us of sem resets + cross-core barrier) runs after
    the end block, so the transfers complete long before the NEFF exits.
    Strip the stores' semaphore updates (so the epilogue's sem-file reset
    cannot race a late increment) and cap every wait on those lanes to the
    count still reachable from the remaining increments."""
    f = nc.m.functions[0]
    body = next(
        (bb for bb in f.blocks if "tile_context" in bb.name
         and not bb.name.endswith("_end")),
        None,
    )
    if body is None:
        return
    import bass_rust as br

    # The accumulator-store DMAs are emitted last in the body block.
    stores = [
        i for i in body.instructions if type(i).__name__ == "InstDMACopy"
    ][-n_stores:]
    stripped = {}
    for inst in stores:
        si = inst.sync_info
        if si is not None and si.on_update:
            zeroed = []
            for u in si.on_update:
                stripped[u.id] = stripped.get(u.id, 0) + (u.update_value or 0)
                zeroed.append(
                    br.SyncUpdate(
                        sync_type=u.sync_type,
                        id=u.id,
                        ant_name=u.ant_name,
                        update_mode=u.update_mode,
                        update_value=0,
                        update_reg=u.update_reg,
                    )
                )
            inst.sync_info = type(si)(
                on_wait=list(si.on_wait), on_update=zeroed
            )
    if not stripped:
        return
    # Final reachable count per sem = old final - stripped (the zeroed
    # updates no longer contribute). Tile's waits use absolute sem-ge-imm
    # values, so cap any wait above the new final.
    finals = {}
    for bb in f.blocks:
        for inst in bb.instructions:
            si = inst.sync_info
            if si is None:
                continue
            for u in si.on_update or []:
                if u.id in stripped:
                    finals[u.id] = finals.get(u.id, 0) + (u.update_value or 0)

    for bb in f.blocks:
        for inst in bb.instructions:
            si = inst.sync_info
            if si is None or not si.on_wait:
                continue
            if not any(
                w.id in stripped
                and w.wait_value is not None
                and w.wait_value > finals.get(w.id, 0)
                for w in si.on_wait
            ):
                continue
            new_waits = []
            for w in si.on_wait:
                if (
                    w.id in stripped
                    and w.wait_value is not None
                    and w.wait_value > finals.get(w.id, 0)
                ):
                    new_waits.append(
                        br.SyncWait(
                            sync_type=w.sync_type,
                            id=w.id,
                            ant_name=w.ant_name,
                            wait_mode=w.wait_mode,
                            wait_value=finals.get(w.id, 0),
                            wait_reg=w.wait_reg,
                        )
                    )
                else:
                    new_waits.append(w)
            inst.sync_info = type(si)(
                on_wait=new_waits, on_update=list(si.on_update)
            )


N_CORES = 8
B, C, D, H, W = 2, 2, 32, 512, 512
GROUPS = N_CORES // B          # 4 z-groups per batch
DG = D // GROUPS               # 8 z-slices per core
P = 128                        # SBUF partitions
TOTAL_W = DG * H * W // P      # 16384 free-dim elements per partition
PLANE = TOTAL_W * P            # elements per (core, channel) plane

# Per-partition chunk widths. Bigger leading chunks cut per-instruction
# and per-event overhead; the tapered tail keeps the post-last-DMA
# compute drain short. The last N_TAIL chunks form the separate PE
# accumulation group whose store happens at the very end.
WIDTHS = [1024, 2048, 4096, 4096, 4096, 768, 256]
N_TAIL = 2
# chunks whose p1^2 square runs on DVE+PE instead of ACT (~25% of the
# plane): drops ACT busy from ~34.6us to ~31us, matching DVE's ~30us
DVE_SQ1_CHUNKS = frozenset({4})
assert sum(WIDTHS) == TOTAL_W

_nc_cache = {}
last_results = None


def build_nc(widths=WIDTHS, n_tail=N_TAIL, dve_sq1=DVE_SQ1_CHUNKS):
    """Per-core SPMD program: partial sums for one shard."""
    f32, i8 = mybir.dt.float32, mybir.dt.int8
    bf16 = mybir.dt.bfloat16
    alu = mybir.AluOpType
    act = mybir.ActivationFunctionType

    chunks, pos = [], 0
    for w in widths:
        chunks.append((pos, w))
        pos += P * w
    assert pos == PLANE
    nch = len(chunks)
    n_bulk = nch - n_tail

    nc = bass.Bass(enable_partition_id=False)
    # p is chunk-pair interleaved by the host: for each chunk, the p0
    # block [P, w] then the p1 block [P, w], column-concatenated per
    # partition, so one contiguous DMA delivers both channels.
    p = nc.dram_tensor("p", [2 * PLANE], bf16, kind="ExternalInput")
    m = nc.dram_tensor("m", [PLANE], i8, kind="ExternalInput")
    # ACT accum columns per chunk k: 2k = sum((p0-1)^2), 2k+1 = sum(p1^2)
    out = nc.dram_tensor("out", [P, 2 * nch], f32, kind="ExternalOutput")
    # DVE accum columns per chunk k: sum(t*q). Separate tensor so the two
    # engines never share an accumulator tile (avoids cross-engine
    # serialization via tile-dependency tracking).
    outv = nc.dram_tensor("outv", [P, nch], f32, kind="ExternalOutput")
    # PE-reduced sum(p1^2) partials for the DVE_SQ1 chunks: [512] f32
    out2 = nc.dram_tensor("out2", [512], f32, kind="ExternalOutput")

    ones = nc.const_aps.aps[(bf16, 1.0)]     # [128, 1] SBUF constant

    def chunk_ap(t, base, start, w):
        return t[base + start : base + start + P * w].rearrange(
            "(p w) -> p w", p=P
        )

    with TileContext(nc) as tc:
        with (
            tc.tile_pool(name="acc", bufs=1) as acc_pool,
            tc.tile_pool(name="mp", bufs=4) as m_pool,
            tc.tile_pool(name="pp", bufs=5) as p_pool,
            tc.tile_pool(name="qp", bufs=3) as q_pool,
            tc.tile_pool(name="sv", bufs=3) as s_dve_pool,
            tc.tile_pool(name="sap", bufs=3) as s_act_pool,
            tc.tile_pool(name="wrm", bufs=1) as warm_pool,
            tc.psum_pool(name="ps", bufs=1) as psum_pool,
            tc.tile_pool(name="fin", bufs=1) as fin_pool,
        ):
            acc = acc_pool.tile([P, 2 * nch], f32)
            accv = acc_pool.tile([P, nch], f32)
            ps_s1 = psum_pool.tile([1, 512], f32)
            n_mm = sum((w + 511) // 512 for k, w in enumerate(widths)
                       if k in dve_sq1)
            mm_done = [0]

            # ACT warmup: load the Square table before any data lands so
            # the 1.3us table load overlaps the first input DMAs.
            warm = warm_pool.tile([P, 1], bf16)
            nc.scalar.activation(warm[:], ones, act.Square)

            for k, (start, w) in enumerate(chunks):
                # One hardware-DGE (Sync) DMA per chunk for both prob
                # channels (host interleaved them) plus one for the mask.
                # Software-DGE (gpsimd) is avoided entirely: its transfers
                # measured both slower per byte and late to start.
                pt = p_pool.tile([P, 2 * w], bf16, tag="pt")
                half = P * w
                nc.sync.dma_start(
                    pt[:, :w],
                    p[2 * start : 2 * start + half].rearrange(
                        "(p w) -> p w", p=P
                    ),
                )
                nc.sync.dma_start(
                    pt[:, w:],
                    p[2 * start + half : 2 * start + 2 * half].rearrange(
                        "(p w) -> p w", p=P
                    ),
                )
                mt = m_pool.tile([P, w], i8, tag="mt")
                nc.sync.dma_start(mt[:], chunk_ap(m, 0, start, w))
                pt0, pt1 = pt[:, :w], pt[:, w:]
                # ACT: acc[3k] = sum((1-p0)^2) = sum((p0-1)^2)
                sq0 = s_act_pool.tile([P, w], bf16, tag="sq")
                nc.scalar.activation(
                    sq0[:], pt0, act.Square, bias=1.0, scale=-1.0,
                    accum_out=acc[:, 2 * k : 2 * k + 1],
                )
                if k in dve_sq1:
                    # offloaded p1^2: DVE multiply (2x) + PE ones-matmul
                    # reduction into PSUM on otherwise idle engines
                    sqd = s_dve_pool.tile([P, w], bf16, tag="tq")
                    nc.vector.tensor_tensor(sqd[:], pt1, pt1, op=alu.mult)
                    g0 = 0
                    while g0 < w:
                        gw = min(512, w - g0)
                        mm_done[0] += 1
                        nc.tensor.matmul(
                            ps_s1[:, :gw], ones, sqd[:, g0 : g0 + gw],
                            start=mm_done[0] == 1,
                            stop=mm_done[0] == n_mm,
                        )
                        g0 += gw
                    if mm_done[0] == n_mm:
                        fin = fin_pool.tile([1, 512], f32)
                        nc.vector.tensor_copy(fin[:], ps_s1[:, :])
                        nc.sync.dma_start(
                            out2[:].rearrange("(p w) -> p w", p=1), fin[:]
                        )
                else:
                    # ACT: acc[2k+1] = sum(p1^2)
                    sq1 = s_act_pool.tile([P, w], bf16, tag="sq")
                    nc.scalar.activation(
                        sq1[:], pt1, act.Square,
                        accum_out=acc[:, 2 * k + 1 : 2 * k + 2],
                    )
                # DVE: q = p0 - p1 (2x), then one fused pass
                # acc[3k+2] = sum((m==1) * q), reading the int8 mask
                qt = q_pool.tile([P, w], bf16, tag="qt")
                nc.vector.tensor_tensor(qt[:], pt0, pt1, op=alu.subtract)
                tq = s_dve_pool.tile([P, w], bf16, tag="tq")
                nc.vector.scalar_tensor_tensor(
                    tq[:], mt[:], 1.0, qt[:],
                    op0=alu.is_equal, op1=alu.mult,
                    accum_out=accv[:, k : k + 1],
                )
                if k == n_bulk - 1:
                    # ship finished accumulator columns while the tail
                    # chunks still compute
                    nc.sync.dma_start(
                        out[:, : 2 * n_bulk], acc[:, : 2 * n_bulk]
                    )
                    nc.sync.dma_start(outv[:, :n_bulk], accv[:, :n_bulk])
            nc.sync.dma_start(out[:, 2 * n_bulk :], acc[:, 2 * n_bulk :])
            nc.sync.dma_start(outv[:, n_bulk:], accv[:, n_bulk:])
    split_multiwait_instructions(nc)
    hoist_leading_dmas(nc)
    overlap_final_store(nc, n_stores=2)  # the two tail stores
    nc.finalize()
    return nc


def _get_nc():
    if "nc" not in _nc_cache:
        _nc_cache["nc"] = build_nc()
    return _nc_cache["nc"]


def shard_inputs(probs, gt_mask, widths=WIDTHS):
    import ml_dtypes

    pb = probs.astype(ml_dtypes.bfloat16)    # (B,C,D,H,W) bf16
    mb = gt_mask.astype(np.int8)             # (B,D,H,W) i8, values {0,1,2}
    in_maps = []
    for k in range(N_CORES):
        b, g = divmod(k, GROUPS)
        z0 = g * DG
        # chunk-pair interleave: per chunk, p0 cols then p1 cols, so the
        # device reads both channels in one contiguous DMA
        p0 = pb[b, 0, z0 : z0 + DG].reshape(P, TOTAL_W)
        p1 = pb[b, 1, z0 : z0 + DG].reshape(P, TOTAL_W)
        arr = np.empty((P, 2 * TOTAL_W), dtype=pb.dtype)
        c = 0
        for w in widths:
            arr[:, 2 * c : 2 * c + w] = p0[:, c : c + w]
            arr[:, 2 * c + w : 2 * (c + w)] = p1[:, c : c + w]
            c += w
        in_maps.append(
            {
                "p": arr.reshape(-1),
                "m": mb[b, z0 : z0 + DG].reshape(-1),
            }
        )
    return in_maps


def kernel(probs, gt_mask):
    global last_results
    probs = np.ascontiguousarray(probs, dtype=np.float32)
    gt_mask = np.ascontiguousarray(gt_mask, dtype=np.int32)
    assert probs.shape == (B, C, D, H, W) and gt_mask.shape == (B, D, H, W)

    nc = _get_nc()
    in_maps = shard_inputs(probs, gt_mask)
    trace = bool(os.environ.get("BETTI_TRACE"))
    last_results = run_bass_kernel_spmd(
        nc, in_maps, core_ids=list(range(N_CORES)), trace=trace
    )
    total = 0.0
    for r in last_results.results:
        a = r["out"].astype(np.float64)       # [P, 2*nch] ACT squares
        v = r["outv"].astype(np.float64)      # [P, nch]   DVE sum(t*q)
        s1 = r["out2"].astype(np.float64)     # [512] PE sum(p1^2) partial
        total += a.sum() + 2.0 * v.sum() + s1.sum()
    return np.asarray(total / (B * C * D * H * W), dtype=np.float32)


# revision 23
# speedup vs baseline: 1.0330x; 1.0330x over previous
"""Betti-matching surrogate loss kernel for Trainium2 (8 NeuronCores).

Computes mean((probs - one_hot(gt_mask))^2) where gt_mask values are
{0,1,2} with ignore_index 2 mapped to class 0 (so class = (gt_mask == 1)).

Identity used (t := (m==1) in {0,1}):

    loss * N = sum((p0-1)^2) + sum(p1^2) + 2*sum(t * (p0 - p1))

HBM traffic is the roofline for this problem, so the host narrows
dtypes while sharding: probs f32 -> bf16 (device compute is bf16
anyway; the loss shifts ~5e-5 relative), gt_mask int32 -> int8
(lossless). Per-core bytes drop 24 MiB -> 10 MiB. Note the DMA engines
charge OUTPUT bytes, so fp8-with-cast-on-DMA does not beat bf16 here
(measured); 10 MiB of SBUF-side bytes (~29 us) is the floor for ops
that need 2-byte operands.

Engine split, chosen from measured rates (ACT pass 13.7us/plane any
dtype, DVE tensor_tensor 2x 8.5us/plane, DVE scalar_tensor_tensor 1x
but fused compare+mult+accumulate in one pass):

  ACT: acc0 = Square(1-p0) accumulate; acc1 = Square(p1) accumulate
       (all but one 4096-wide chunk)
  DVE: q = p0-p1 (2x); acc2 = sum((m==1)*q) via one fused
       scalar_tensor_tensor pass reading the int8 mask directly;
       p1*p1 for the offloaded chunk
  PE : ones-matmul reduction of the offloaded p1^2 chunk into PSUM

All input DMAs are hardware-DGE on the Sync queue (software DGE via
gpsimd measured slower per byte and its hoisted transfers stall the
init barrier). The first chunks' DMAs and the activation-table warmup
are hoisted into the entry block so the stream and the 1.3us Square
table load run during the framework preamble. Engine busy lands at
~30.5us each for ACT/DVE with the 10 MiB stream at ~29us; measured
span is that plus ~6us fixed preamble, ~2.5us end-block, and a few us
of pipeline fill/drain.

Sharding: core k = (b, g) with b = k // 4, g = k % 4 owns
probs[b, :, 8g:8g+8, :, :] and gt_mask[b, 8g:8g+8, :, :] — contiguous
views of the dtype-narrowed full inputs. Host reduces partials in f64.
"""

import os

import numpy as np

import concourse.bass as bass
import concourse.mybir as mybir
from concourse.bass_utils import run_bass_kernel_spmd
from concourse.tile import TileContext


import bass_rust


def split_multiwait_instructions(nc):
    """The walrus build in this image rejects any instruction carrying more
    than one sync wait ("Too many sync wait commands"). Tile's semaphore
    assignment freely attaches several. Hoist all but the last wait of each
    instruction onto injected same-engine NoOps placed directly before it —
    engine streams execute in order, so the waits still all complete before
    the real instruction issues."""
    k = 0
    for f in nc.m.functions:
        for bb in f.blocks:
            insts = bb.instructions
            out, changed = [], False
            for inst in insts:
                si = inst.sync_info
                if si is not None and si.on_wait and len(si.on_wait) > 1:
                    SI = type(si)
                    waits = list(si.on_wait)
                    for w in waits[:-1]:
                        nop = bass_rust.InstNoOp(
                            name=f"waitsplit-{k}",
                            engine=inst.engine,
                            sync_info=SI(on_wait=[w], on_update=[]),
                        )
                        k += 1
                        nc.register_instruction(nop)
                        out.append(nop)
                    inst.sync_info = SI(
                        on_wait=[waits[-1]], on_update=list(si.on_update)
                    )
                    changed = True
                out.append(inst)
            if changed:
                bb.instructions = out

def hoist_leading_dmas(nc, max_hoist=4):
    """Launch the input stream during the framework preamble: move the
    leading wait-free Sync-queue DMACopy instructions out of the body
    block and into the entry block, ahead of the init-barrier Drain.
    The SP sequencer dispatches them asynchronously before joining the
    barrier, so the transfers overlap the const-memset/barrier preamble.
    Only hardware-DGE (SP) DMAs are eligible: a gpsimd software-DGE DMA
    in main stalls the init barrier's Drain until the transfer itself
    completes (measured 7.7us). Also hoists the leading wait-free
    Activation (the Square-table warmup) so the 1.3us table load runs
    during the preamble."""
    f = nc.m.functions[0]
    blocks = {bb.name: bb for bb in f.blocks}
    body = next(
        (bb for bb in f.blocks if "tile_context" in bb.name
         and not bb.name.endswith("_end")),
        None,
    )
    main = blocks.get("main")
    if body is None or main is None:
        return
    hoist = []
    n_dma = 0
    for inst in body.instructions:
        tn = type(inst).__name__
        has_wait = inst.sync_info is not None and inst.sync_info.on_wait
        if tn == "InstDMACopy" and str(inst.engine) in ("EngineType.SP", "SP"):
            if has_wait or n_dma >= max_hoist:
                break
            hoist.append(inst)
            n_dma += 1
        elif tn == "InstActivation" and not has_wait and not hoist:
            hoist.append(inst)
        elif tn in ("InstDMACopy", "InstNoOp"):
            continue
        else:
            break
    if not hoist:
        return
    names = {i.name for i in hoist}
    body.instructions = [i for i in body.instructions if i.name not in names]
    mi = main.instructions
    # Insert right after the entry InstCall: the SP sequencer then issues
    # the DMAs before its register moves, pulling the stream start forward.
    cut = 1 if mi and type(mi[0]).__name__ == "InstCall" else 0
    main.instructions = mi[:cut] + hoist + mi[cut:]


def overlap_final_store(nc, n_stores=2):
    """Take the output-store DMAs' HBM-write receipt off the critical path.
    The kernel tail otherwise serializes: last compute -> store DMA issue ->
    ~1.4us sem-update receipt -> end-block waits -> barriers -> epilogue.
    Nothing in the program consumes the stores' data or slots, and the
    wrapper epilogue (~7us of sem resets + cross-core barrier) runs after
    the end block, so the transfers complete long before the NEFF exits.
    Strip the stores' semaphore updates (so the epilogue's sem-file reset
    cannot race a late increment) and cap every wait on those lanes to the
    count still reachable from the remaining increments."""
    f = nc.m.functions[0]
    body = next(
        (bb for bb in f.blocks if "tile_context" in bb.name
         and not bb.name.endswith("_end")),
        None,
    )
    if body is None:
        return
    import bass_rust as br

    # The accumulator-store DMAs are emitted last in the body block.
    stores = [
        i for i in body.instructions if type(i).__name__ == "InstDMACopy"
    ][-n_stores:]
    stripped = {}
    for inst in stores:
        si = inst.sync_info
        if si is not None and si.on_update:
            zeroed = []
            for u in si.on_update:
                stripped[u.id] = stripped.get(u.id, 0) + (u.update_value or 0)
                zeroed.append(
                    br.SyncUpdate(
                        sync_type=u.sync_type,
                        id=u.id,
                        ant_name=u.ant_name,
                        update_mode=u.update_mode,
                        update_value=0,
                        update_reg=u.update_reg,
                    )
                )
            inst.sync_info = type(si)(
                on_wait=list(si.on_wait), on_update=zeroed
            )
    if not stripped:
        return
    # Final reachable count per sem = old final - stripped (the zeroed
    # updates no longer contribute). Tile's waits use absolute sem-ge-imm
    # values, so cap any wait above the new final.
    finals = {}
    for bb in f.blocks:
        for inst in bb.instructions:
            si = inst.sync_info
            if si is None:
                continue
            for u in si.on_update or []:
                if u.id in stripped:
                    finals[u.id] = finals.get(u.id, 0) + (u.update_value or 0)

    for bb in f.blocks:
        for inst in bb.instructions:
            si = inst.sync_info
            if si is None or not si.on_wait:
                continue
            if not any(
                w.id in stripped
                and w.wait_value is not None
                and w.wait_value > finals.get(w.id, 0)
                for w in si.on_wait
            ):
                continue
            new_waits = []
            for w in si.on_wait:
                if (
                    w.id in stripped
                    and w.wait_value is not None
                    and w.wait_value > finals.get(w.id, 0)
                ):
                    new_waits.append(
                        br.SyncWait(
                            sync_type=w.sync_type,
                            id=w.id,
                            ant_name=w.ant_name,
                            wait_mode=w.wait_mode,
                            wait_value=finals.get(w.id, 0),
                            wait_reg=w.wait_reg,
                        )
                    )
                else:
                    new_waits.append(w)
            inst.sync_info = type(si)(
                on_wait=new_waits, on_update=list(si.on_update)
            )


N_CORES = 8
B, C, D, H, W = 2, 2, 32, 512, 512
GROUPS = N_CORES // B          # 4 z-groups per batch
DG = D // GROUPS               # 8 z-slices per core
P = 128                        # SBUF partitions
TOTAL_W = DG * H * W // P      # 16384 free-dim elements per partition
PLANE = TOTAL_W * P            # elements per (core, channel) plane

# Per-partition chunk widths. Bigger leading chunks cut per-instruction
# and per-event overhead; the tapered tail keeps the post-last-DMA
# compute drain short. The last N_TAIL chunks form the separate PE
# accumulation group whose store happens at the very end.
WIDTHS = [1024, 2048, 4096, 4096, 4096, 768, 256]
N_TAIL = 2
# chunks whose p1^2 square runs on DVE+PE instead of ACT (~25% of the
# plane): drops ACT busy from ~34.6us to ~31us, matching DVE's ~30us
DVE_SQ1_CHUNKS = frozenset({4})
assert sum(WIDTHS) == TOTAL_W

_nc_cache = {}
last_results = None


def build_nc(widths=WIDTHS, n_tail=N_TAIL, dve_sq1=DVE_SQ1_CHUNKS):
    """Per-core SPMD program: partial sums for one shard."""
    f32, i8 = mybir.dt.float32, mybir.dt.int8
    bf16 = mybir.dt.bfloat16
    alu = mybir.AluOpType
    act = mybir.ActivationFunctionType

    chunks, pos = [], 0
    for w in widths:
        chunks.append((pos, w))
        pos += P * w
    assert pos == PLANE
    nch = len(chunks)
    n_bulk = nch - n_tail

    nc = bass.Bass(enable_partition_id=False)
    # p is chunk-pair interleaved by the host: for each chunk, the p0
    # block [P, w] then the p1 block [P, w], column-concatenated per
    # partition, so one contiguous DMA delivers both channels.
    p = nc.dram_tensor("p", [2 * PLANE], bf16, kind="ExternalInput")
    m = nc.dram_tensor("m", [PLANE], i8, kind="ExternalInput")
    # ACT accum columns per chunk k: 2k = sum((p0-1)^2), 2k+1 = sum(p1^2)
    out = nc.dram_tensor("out", [P, 2 * nch], f32, kind="ExternalOutput")
    # DVE accum columns per chunk k: sum(t*q). Separate tensor so the two
    # engines never share an accumulator tile (avoids cross-engine
    # serialization via tile-dependency tracking).
    outv = nc.dram_tensor("outv", [P, nch], f32, kind="ExternalOutput")
    # PE-reduced sum(p1^2) partials for the DVE_SQ1 chunks: [512] f32
    out2 = nc.dram_tensor("out2", [512], f32, kind="ExternalOutput")

    ones = nc.const_aps.aps[(bf16, 1.0)]     # [128, 1] SBUF constant

    def chunk_ap(t, base, start, w):
        return t[base + start : base + start + P * w].rearrange(
            "(p w) -> p w", p=P
        )

    with TileContext(nc) as tc:
        with (
            tc.tile_pool(name="acc", bufs=1) as acc_pool,
            tc.tile_pool(name="mp", bufs=4) as m_pool,
            tc.tile_pool(name="pp", bufs=5) as p_pool,
            tc.tile_pool(name="qp", bufs=3) as q_pool,
            tc.tile_pool(name="sv", bufs=3) as s_dve_pool,
            tc.tile_pool(name="sap", bufs=3) as s_act_pool,
            tc.tile_pool(name="wrm", bufs=1) as warm_pool,
            tc.psum_pool(name="ps", bufs=1) as psum_pool,
            tc.tile_pool(name="fin", bufs=1) as fin_pool,
        ):
            acc = acc_pool.tile([P, 2 * nch], f32)
            accv = acc_pool.tile([P, nch], f32)
            ps_s1 = psum_pool.tile([1, 512], f32)
            n_mm = sum((w + 511) // 512 for k, w in enumerate(widths)
                       if k in dve_sq1)
            mm_done = [0]

            # ACT warmup: load the Square table before any data lands so
            # the 1.3us table load overlaps the first input DMAs.
            warm = warm_pool.tile([P, 1], bf16)
            nc.scalar.activation(warm[:], ones, act.Square)

            for k, (start, w) in enumerate(chunks):
                # One hardware-DGE (Sync) DMA per chunk for both prob
                # channels (host interleaved them) plus one for the mask.
                # Software-DGE (gpsimd) is avoided entirely: its transfers
                # measured both slower per byte and late to start.
                pt = p_pool.tile([P, 2 * w], bf16, tag="pt")
                half = P * w
                nc.sync.dma_start(
                    pt[:, :w],
                    p[2 * start : 2 * start + half].rearrange(
                        "(p w) -> p w", p=P
                    ),
                )
                nc.sync.dma_start(
                    pt[:, w:],
                    p[2 * start + half : 2 * start + 2 * half].rearrange(
                        "(p w) -> p w", p=P
                    ),
                )
                mt = m_pool.tile([P, w], i8, tag="mt")
                nc.sync.dma_start(mt[:], chunk_ap(m, 0, start, w))
                pt0, pt1 = pt[:, :w], pt[:, w:]
                # ACT: acc[3k] = sum((1-p0)^2) = sum((p0-1)^2)
                sq0 = s_act_pool.tile([P, w], bf16, tag="sq")
                nc.scalar.activation(
                    sq0[:], pt0, act.Square, bias=1.0, scale=-1.0,
                    accum_out=acc[:, 2 * k : 2 * k + 1],
                )
                if k in dve_sq1:
                    # offloaded p1^2: DVE multiply (2x) + PE ones-matmul
                    # reduction into PSUM on otherwise idle engines
                    sqd = s_dve_pool.tile([P, w], bf16, tag="tq")
                    nc.vector.tensor_tensor(sqd[:], pt1, pt1, op=alu.mult)
                    g0 = 0
                    while g0 < w:
                        gw = min(512, w - g0)
                        mm_done[0] += 1
                        nc.tensor.matmul(
                            ps_s1[:, :gw], ones, sqd[:, g0 : g0 + gw],
                            start=mm_done[0] == 1,
                            stop=mm_done[0] == n_mm,
                        )
                        g0 += gw
                    if mm_done[0] == n_mm:
                        fin = fin_pool.tile([1, 512], f32)
                        nc.vector.tensor_copy(fin[:], ps_s1[:, :])
                        nc.sync.dma_start(
                            out2[:].rearrange("(p w) -> p w", p=1), fin[:]
                        )
                else:
                    # ACT: acc[2k+1] = sum(p1^2)
                    sq1 = s_act_pool.tile([P, w], bf16, tag="sq")
                    nc.scalar.activation(
                        sq1[:], pt1, act.Square,
                        accum_out=acc[:, 2 * k + 1 : 2 * k + 2],
                    )
                # DVE: q = p0 - p1 (2x), then one fused pass
                # acc[3k+2] = sum((m==1) * q), reading the int8 mask
                qt = q_pool.tile([P, w], bf16, tag="qt")
                nc.vector.tensor_tensor(qt[:], pt0, pt1, op=alu.subtract)
                tq = s_dve_pool.tile([P, w], bf16, tag="tq")
                nc.vector.scalar_tensor_tensor(
                    tq[:], mt[:], 1.0, qt[:],
                    op0=alu.is_equal, op1=alu.mult,
                    accum_out=accv[:, k : k + 1],
                )
                if k == n_bulk - 1:
                    # ship finished accumulator columns while the tail
                    # chunks still compute
                    nc.sync.dma_start(
                        out[:, : 2 * n_bulk], acc[:, : 2 * n_bulk]
                    )
                    nc.sync.dma_start(outv[:, :n_bulk], accv[:, :n_bulk])
            nc.sync.dma_start(out[:, 2 * n_bulk :], acc[:, 2 * n_bulk :])
            nc.sync.dma_start(outv[:, n_bulk:], accv[:, n_bulk:])
    split_multiwait_instructions(nc)
    hoist_leading_dmas(nc)
    overlap_final_store(nc, n_stores=2)  # the two tail stores
    nc.finalize()
    return nc


def _get_nc():
    if "nc" not in _nc_cache:
        _nc_cache["nc"] = build_nc()
    return _nc_cache["nc"]


def shard_inputs(probs, gt_mask, widths=WIDTHS):
    import ml_dtypes

    pb = probs.astype(ml_dtypes.bfloat16)    # (B,C,D,H,W) bf16
    mb = gt_mask.astype(np.int8)             # (B,D,H,W) i8, values {0,1,2}
    in_maps = []
    for k in range(N_CORES):
        b, g = divmod(k, GROUPS)
        z0 = g * DG
        # chunk-pair interleave: per chunk, p0 cols then p1 cols, so the
        # device reads both channels in one contiguous DMA
        p0 = pb[b, 0, z0 : z0 + DG].reshape(P, TOTAL_W)
        p1 = pb[b, 1, z0 : z0 + DG].reshape(P, TOTAL_W)
        arr = np.empty((P, 2 * TOTAL_W), dtype=pb.dtype)
        c = 0
        for w in widths:
            arr[:, 2 * c : 2 * c + w] = p0[:, c : c + w]
            arr[:, 2 * c + w : 2 * (c + w)] = p1[:, c : c + w]
            c += w
        in_maps.append(
            {
                "p": arr.reshape(-1),
                "m": mb[b, z0 : z0 + DG].reshape(-1),
            }
        )
    return in_maps


def kernel(probs, gt_mask):
    global last_results
    probs = np.ascontiguousarray(probs, dtype=np.float32)
    gt_mask = np.ascontiguousarray(gt_mask, dtype=np.int32)
    assert probs.shape == (B, C, D, H, W) and gt_mask.shape == (B, D, H, W)

    nc = _get_nc()
    in_maps = shard_inputs(probs, gt_mask)
    trace = bool(os.environ.get("BETTI_TRACE"))
    last_results = run_bass_kernel_spmd(
        nc, in_maps, core_ids=list(range(N_CORES)), trace=trace
    )
    total = 0.0
    for r in last_results.results:
        a = r["out"].astype(np.float64)       # [P, 2*nch] ACT squares
        v = r["outv"].astype(np.float64)      # [P, nch]   DVE sum(t*q)
        s1 = r["out2"].astype(np.float64)     # [512] PE sum(p1^2) partial
        total += a.sum() + 2.0 * v.sum() + s1.sum()
    return np.asarray(total / (B * C * D * H * W), dtype=np.float32)
